# revision 26
# baseline (speedup 1.0000x reference)
"""Trainium2 Bass kernel for nn_EnergyModel — fp8(e4m3), range-mask gather.

Only poses with T[:,4:7] inside `ranges` need computing (the rest output the
constant 100000.0, independent of the big tensors) — with randn T that is
~32% of poses.  The host gathers the unmasked poses, folds
c[q,d] = 16*sqrt(2 a_q w_d) into both tensors, quantizes to float8_e4m3, and
remaps features so SBUF partition = f mod 128 and poses pack densely along
the free axis: per core [128, n_poses * 576], chunked ppc poses at a time as
[x-cols | y-cols] halves.

Per chunk (C = ppc*576 cols):
  cols [0, 5120):  TensorE DoubleRow subtract (S=[I|-I]) -> f32 PSUM
  cols [5120, C):  DVE tensor_tensor subtract (fp8e4 -> bf16 SBUF)
  squares (elementwise, no accumulate): ScalarE Square on the PSUM part +
  tail; GPSIMD tensor_tensor mult on 1024 SBUF cols -> one bf16 sq tile
  per-pose energies: DVE segmented tensor_reduce [128, ppc, 576] -> A[:, ...]
  (software-pipelined one chunk behind the squares)
Cross-partition finish: one f32 matmul ones(*2^-8)^T @ A -> [1, n] energies.
"""

import sys

import numpy as np
import ml_dtypes

for _p in ("/opt/trn_rl_repo",):
    if _p not in sys.path:
        sys.path.insert(0, _p)

import concourse.bacc as bacc
import concourse.bass as bass
import concourse.mybir as mybir
from concourse.bass_utils import run_bass_kernel_spmd
from concourse.tile import TileContext

N_CORES = 8
NT, NQ, D = 1024, 128, 576
G = 192
LN2 = 0.6931471805599453
F_TOT = NQ * D
BUMP = 16.0
S_DIM = 576  # feature sub-chunks per partition: f = s*128 + p

_GROUP_DIMS = np.array([1] * 64 + [3] * 64 + [5] * 64)

_cache: dict = {}
_last_in_maps: list | None = None

PUNIT = 1024  # PSUM tile width


def _build(ppc: int, nchunks: int, bufs: int = 3) -> bass.Bass:
    C = ppc * S_DIM
    n_c = ppc * nchunks  # poses per core (padded)
    pe_cols = min(5120, (C // PUNIT) * PUNIT)
    npunits = pe_cols // PUNIT
    dve_cols = C - pe_cols
    gp_cols = min(512, dve_cols)
    f32 = mybir.dt.float32
    bf16 = mybir.dt.bfloat16
    f8 = mybir.dt.float8e4

    nc = bacc.Bacc(
        "TRN2", target_bir_lowering=False, debug=False, num_devices=N_CORES
    )
    zin = nc.declare_dram_parameter(
        "zin", [128, nchunks * 2 * C], f8, isOutput=False
    )
    smat = nc.declare_dram_parameter("smat", [128, 2 * 128], f8, isOutput=False)
    onesv = nc.declare_dram_parameter("onesv", [128, 1], f32, isOutput=False)
    energy = nc.declare_dram_parameter("energy", [1, n_c], f32, isOutput=True)

    with TileContext(nc) as tc:
        with (
            tc.tile_pool(name="io", bufs=bufs) as io,
            tc.tile_pool(name="sq", bufs=2) as sqp,
            tc.tile_pool(name="df", bufs=2) as df,
            tc.tile_pool(name="ps", bufs=3, space="PSUM") as ps,
            tc.tile_pool(name="pe", bufs=1, space="PSUM") as pe_pool,
            tc.tile_pool(name="acc", bufs=1) as acc,
        ):
            z0 = io.tile([128, 2 * C], f8, tag="z")
            nc.sync.dma_start(out=z0[:, : 2 * dve_cols], in_=zin[:, : 2 * dve_cols])
            nc.sync.dma_start(out=z0[:, 2 * dve_cols :], in_=zin[:, 2 * dve_cols : 2 * C])
            s_t = acc.tile([128, 2 * 128], f8)
            nc.sync.dma_start(out=s_t[:], in_=smat[:])
            sview = s_t[:].rearrange("p (two f) -> p two f", two=2)
            ones_t = acc.tile([128, 1], f32)
            nc.sync.dma_start(out=ones_t[:], in_=onesv[:])
            A = acc.tile([128, n_c], f32)

            sq_tiles = []
            for c in range(nchunks):
                if c == 0:
                    z_t = z0
                else:
                    z_t = io.tile([128, 2 * C], f8, tag="z")
                    nc.sync.dma_start(
                        out=z_t[:, : 2 * dve_cols],
                        in_=zin[:, c * 2 * C : c * 2 * C + 2 * dve_cols],
                    )
                    nc.sync.dma_start(
                        out=z_t[:, 2 * dve_cols :],
                        in_=zin[:, c * 2 * C + 2 * dve_cols : (c + 1) * 2 * C],
                    )
                zB = z_t[:, : 2 * dve_cols].rearrange("p (two f) -> p two f", two=2)
                zv = z_t[:, 2 * dve_cols :].rearrange("p (two f) -> p two f", two=2)
                sq_t = sqp.tile([128, C], bf16, tag="s")

                # DVE subtract for the dve_cols block (arrives first)
                if dve_cols > 0:
                    diff = df.tile([128, dve_cols], bf16, tag="d")
                    nc.vector.tensor_tensor(
                        diff[:],
                        zB[:, 0, :],
                        zB[:, 1, :],
                        mybir.AluOpType.subtract,
                    )

                # PE subtract -> PSUM, ScalarE squares -> sq tile
                for u in range(npunits):
                    base = u * PUNIT
                    pt = ps.tile([128, PUNIT], f32, tag="p")
                    for k in range(PUNIT // 512):
                        nc.tensor.matmul(
                            out=pt[:, k * 512 : (k + 1) * 512],
                            lhsT=sview,
                            rhs=zv[:, :, base + k * 512 : base + (k + 1) * 512],
                            start=True,
                            stop=True,
                            perf_mode=mybir.MatmulPerfMode.DoubleRow,
                        )
                    nc.scalar.activation(
                        sq_t[:, base : base + PUNIT],
                        pt[:],
                        mybir.ActivationFunctionType.Square,
                        bias=0.0,
                        scale=1.0,
                    )

                if dve_cols > 0:
                    # GPSIMD squares gp_cols of the SBUF diff
                    nc.gpsimd.tensor_tensor(
                        sq_t[:, pe_cols : pe_cols + gp_cols],
                        diff[:, :gp_cols],
                        diff[:, :gp_cols],
                        mybir.AluOpType.mult,
                    )
                    if gp_cols < dve_cols:
                        nc.scalar.activation(
                            sq_t[:, pe_cols + gp_cols : C],
                            diff[:, gp_cols:],
                            mybir.ActivationFunctionType.Square,
                            bias=0.0,
                            scale=1.0,
                        )

                sq_tiles.append(sq_t)
                # software-pipelined segmented reduce (one chunk behind)
                if c > 0:
                    prev = sq_tiles[c - 1]
                    nc.vector.tensor_reduce(
                        A[:, (c - 1) * ppc : c * ppc],
                        prev[:].rearrange("p (k s) -> p k s", k=ppc),
                        axis=mybir.AxisListType.X,
                        op=mybir.AluOpType.add,
                    )

            nc.vector.tensor_reduce(
                A[:, (nchunks - 1) * ppc : nchunks * ppc],
                sq_tiles[-1][:].rearrange("p (k s) -> p k s", k=ppc),
                axis=mybir.AxisListType.X,
                op=mybir.AluOpType.add,
            )

            # cross-partition: energy[1, n_c] = (ones*inv2)^T @ A
            e_ps = pe_pool.tile([1, n_c], f32)
            nc.tensor.matmul(
                out=e_ps[:], lhsT=ones_t[:], rhs=A[:], start=True, stop=True
            )
            e_sb = acc.tile([1, n_c], f32)
            nc.vector.tensor_copy(e_sb[:], e_ps[:])
            nc.sync.dma_start(out=energy[:], in_=e_sb[:])
    nc.finalize()
    return nc


def _softplus64(x: np.ndarray) -> np.ndarray:
    x = np.asarray(x, dtype=np.float64)
    return np.log1p(np.exp(-np.abs(x))) + np.maximum(x, 0.0)


def kernel(T, descriptor, query_feature, query_attention, irrep_weight_logit, ranges):
    descriptor = np.asarray(descriptor)
    query_feature = np.asarray(query_feature)
    a = np.maximum(np.asarray(query_attention, dtype=np.float64), 0.0)
    w_group = _softplus64(irrep_weight_logit) / (LN2 * G)
    w_feat = np.repeat(w_group, _GROUP_DIMS)
    c_qd = (BUMP * np.sqrt(2.0 * a[:, None] * w_feat[None, :])).astype(np.float32)

    # range mask: energy of out-of-range poses is the constant 1e5
    X = np.asarray(T, dtype=np.float32)[:, 4:7]
    rg = np.asarray(ranges, dtype=np.float32)
    in_range = np.all((rg[None, :, 1] >= X) & (X >= rg[None, :, 0]), axis=-1)
    idx = np.nonzero(in_range)[0]
    n = len(idx)

    n_c = max(1, -(-n // N_CORES))  # poses per core
    ppc = min(16, max(1, -(-n_c // 4)))  # poses per chunk
    nchunks = -(-n_c // ppc)
    n_c = ppc * nchunks
    n_pad = n_c * N_CORES

    # gather + quantize only the needed poses
    xs = np.zeros((n_pad, F_TOT), dtype=ml_dtypes.float8_e4m3)
    ys = np.zeros((n_pad, F_TOT), dtype=ml_dtypes.float8_e4m3)
    cf = c_qd.reshape(1, F_TOT)
    xs[:n] = np.clip(
        descriptor.reshape(NT, F_TOT)[idx] * cf, -240.0, 240.0
    ).astype(ml_dtypes.float8_e4m3)
    ys[:n] = np.clip(
        query_feature.reshape(NT, F_TOT)[idx] * cf, -240.0, 240.0
    ).astype(ml_dtypes.float8_e4m3)

    # remap: [n_pad, (s,p)] -> per core [p, chunk, (x|y), k, s]
    C = ppc * S_DIM
    xs = xs.reshape(N_CORES, nchunks, ppc, S_DIM, 128)
    ys = ys.reshape(N_CORES, nchunks, ppc, S_DIM, 128)
    z = np.stack([xs, ys], axis=2)  # [cores, chunks, 2, ppc, s, p]
    z = np.ascontiguousarray(np.moveaxis(z, 5, 2))  # [cores, chunks, p, 2, k, s]
    z = z.reshape(N_CORES, nchunks, 128, 2 * C)
    PEC = min(5120, (C // 1024) * 1024)
    z = np.concatenate(
        [z[..., PEC:C], z[..., C + PEC :], z[..., :PEC], z[..., C : C + PEC]],
        axis=-1,
    )
    z = np.ascontiguousarray(np.swapaxes(z, 1, 2)).reshape(
        N_CORES, 128, nchunks * 2 * C
    )

    smat = np.zeros((128, 2, 128), dtype=ml_dtypes.float8_e4m3)
    ii = np.arange(128)
    smat[ii, 0, ii] = 1.0
    smat[ii, 1, ii] = -1.0
    smat = smat.reshape(128, 256)
    onesv = np.full((128, 1), 1.0 / (BUMP * BUMP), dtype=np.float32)

    key = ("mask11", ppc, nchunks)
    nc = _cache.get(key)
    if nc is None:
        nc = _build(ppc, nchunks)
        _cache[key] = nc

    in_maps = [
        {"zin": z[i], "smat": smat, "onesv": onesv} for i in range(N_CORES)
    ]

    global _last_in_maps
    _last_in_maps = in_maps
    res = run_bass_kernel_spmd(nc, in_maps, core_ids=list(range(N_CORES)))
    e_sub = np.concatenate([r["energy"][0] for r in res.results])[:n]

    energy = np.full(NT, 100000.0, dtype=np.float32)
    energy[idx] = e_sub.astype(np.float32)
    return energy


# revision 28
# speedup vs baseline: 1.0261x; 1.0261x over previous
"""Trainium2 Bass kernel for nn_EnergyModel — fp8(e4m3), range-mask gather.

Only poses with T[:,4:7] inside `ranges` need computing (the rest output the
constant 100000.0, independent of the big tensors) — with randn T that is
~32% of poses.  The host gathers the unmasked poses, folds
c[q,d] = 16*sqrt(2 a_q w_d) into both tensors, quantizes to float8_e4m3, and
remaps features so SBUF partition = f mod 128 and poses pack densely along
the free axis: per core [128, n_poses * 576], chunked ppc poses at a time as
[x-cols | y-cols] halves.

Per chunk (C = ppc*576 cols):
  cols [0, 5120):  TensorE DoubleRow subtract (S=[I|-I]) -> f32 PSUM
  cols [5120, C):  DVE tensor_tensor subtract (fp8e4 -> bf16 SBUF)
  squares (elementwise, no accumulate): ScalarE Square on the PSUM part +
  tail; GPSIMD tensor_tensor mult on 1024 SBUF cols -> one bf16 sq tile
  per-pose energies: DVE segmented tensor_reduce [128, ppc, 576] -> A[:, ...]
  (software-pipelined one chunk behind the squares)
Cross-partition finish: one f32 matmul ones(*2^-8)^T @ A -> [1, n] energies.
"""

import sys

import numpy as np
import ml_dtypes

for _p in ("/opt/trn_rl_repo",):
    if _p not in sys.path:
        sys.path.insert(0, _p)

import concourse.bacc as bacc
import concourse.bass as bass
import concourse.mybir as mybir
from concourse.bass_utils import run_bass_kernel_spmd
from concourse.tile import TileContext

N_CORES = 8
NT, NQ, D = 1024, 128, 576
G = 192
LN2 = 0.6931471805599453
F_TOT = NQ * D
BUMP = 16.0
S_DIM = 576  # feature sub-chunks per partition: f = s*128 + p

_GROUP_DIMS = np.array([1] * 64 + [3] * 64 + [5] * 64)

_cache: dict = {}
_last_in_maps: list | None = None

PUNIT = 1024  # PSUM tile width


def _build(ppc: int, nchunks: int, bufs: int = 3) -> bass.Bass:
    C = ppc * S_DIM
    n_c = ppc * nchunks  # poses per core (padded)
    pe_cols = min(5120, (C // PUNIT) * PUNIT)
    npunits = pe_cols // PUNIT
    dve_cols = C - pe_cols
    gp_cols = min(512, dve_cols)
    f32 = mybir.dt.float32
    bf16 = mybir.dt.bfloat16
    f8 = mybir.dt.float8e4

    nc = bacc.Bacc(
        "TRN2", target_bir_lowering=False, debug=False, num_devices=N_CORES
    )
    zin = nc.declare_dram_parameter(
        "zin", [128, nchunks * 2 * C], f8, isOutput=False
    )
    smat = nc.declare_dram_parameter("smat", [128, 2 * 128], f8, isOutput=False)
    onesv = nc.declare_dram_parameter("onesv", [128, 1], f32, isOutput=False)
    energy = nc.declare_dram_parameter("energy", [1, n_c], f32, isOutput=True)

    with TileContext(nc) as tc:
        with (
            tc.tile_pool(name="io", bufs=bufs) as io,
            tc.tile_pool(name="sq", bufs=2) as sqp,
            tc.tile_pool(name="df", bufs=2) as df,
            tc.tile_pool(name="ps", bufs=3, space="PSUM") as ps,
            tc.tile_pool(name="pe", bufs=1, space="PSUM") as pe_pool,
            tc.tile_pool(name="acc", bufs=1) as acc,
        ):
            z0 = io.tile([128, 2 * C], f8, tag="z")
            nc.sync.dma_start(out=z0[:, : 2 * dve_cols], in_=zin[:, : 2 * dve_cols])
            nc.sync.dma_start(out=z0[:, 2 * dve_cols :], in_=zin[:, 2 * dve_cols : 2 * C])
            s_t = acc.tile([128, 2 * 128], f8)
            nc.sync.dma_start(out=s_t[:], in_=smat[:])
            sview = s_t[:].rearrange("p (two f) -> p two f", two=2)
            ones_t = acc.tile([128, 1], f32)
            nc.sync.dma_start(out=ones_t[:], in_=onesv[:])
            A = acc.tile([128, n_c], f32)

            sq_tiles = []
            for c in range(nchunks):
                if c == 0:
                    z_t = z0
                else:
                    z_t = io.tile([128, 2 * C], f8, tag="z")
                    nc.sync.dma_start(
                        out=z_t[:, : 2 * dve_cols],
                        in_=zin[:, c * 2 * C : c * 2 * C + 2 * dve_cols],
                    )
                    nc.sync.dma_start(
                        out=z_t[:, 2 * dve_cols :],
                        in_=zin[:, c * 2 * C + 2 * dve_cols : (c + 1) * 2 * C],
                    )
                zB = z_t[:, : 2 * dve_cols].rearrange("p (two f) -> p two f", two=2)
                zv = z_t[:, 2 * dve_cols :].rearrange("p (two f) -> p two f", two=2)
                sq_t = sqp.tile([128, C], bf16, tag="s")

                # DVE subtract for the dve_cols block (arrives first)
                if dve_cols > 0:
                    diff = df.tile([128, dve_cols], bf16, tag="d")
                    nc.vector.tensor_tensor(
                        diff[:],
                        zB[:, 0, :],
                        zB[:, 1, :],
                        mybir.AluOpType.subtract,
                    )

                # PE subtract -> PSUM, ScalarE squares -> sq tile
                for u in range(npunits):
                    base = u * PUNIT
                    pt = ps.tile([128, PUNIT], f32, tag="p")
                    for k in range(PUNIT // 512):
                        nc.tensor.matmul(
                            out=pt[:, k * 512 : (k + 1) * 512],
                            lhsT=sview,
                            rhs=zv[:, :, base + k * 512 : base + (k + 1) * 512],
                            start=True,
                            stop=True,
                            perf_mode=mybir.MatmulPerfMode.DoubleRow,
                        )
                    nc.scalar.activation(
                        sq_t[:, base : base + PUNIT],
                        pt[:],
                        mybir.ActivationFunctionType.Square,
                        bias=0.0,
                        scale=1.0,
                    )

                if dve_cols > 0:
                    # GPSIMD squares gp_cols of the SBUF diff
                    nc.gpsimd.tensor_tensor(
                        sq_t[:, pe_cols : pe_cols + gp_cols],
                        diff[:, :gp_cols],
                        diff[:, :gp_cols],
                        mybir.AluOpType.mult,
                    )
                    if gp_cols < dve_cols:
                        nc.scalar.activation(
                            sq_t[:, pe_cols + gp_cols : C],
                            diff[:, gp_cols:],
                            mybir.ActivationFunctionType.Square,
                            bias=0.0,
                            scale=1.0,
                        )

                sq_tiles.append(sq_t)
                # software-pipelined segmented reduce (one chunk behind)
                if c > 0:
                    prev = sq_tiles[c - 1]
                    nc.vector.tensor_reduce(
                        A[:, (c - 1) * ppc : c * ppc],
                        prev[:].rearrange("p (k s) -> p k s", k=ppc),
                        axis=mybir.AxisListType.X,
                        op=mybir.AluOpType.add,
                    )

            nc.vector.tensor_reduce(
                A[:, (nchunks - 1) * ppc : nchunks * ppc],
                sq_tiles[-1][:].rearrange("p (k s) -> p k s", k=ppc),
                axis=mybir.AxisListType.X,
                op=mybir.AluOpType.add,
            )

            # cross-partition: energy[1, n_c] = (ones*inv2)^T @ A
            e_ps = pe_pool.tile([1, n_c], f32)
            nc.tensor.matmul(
                out=e_ps[:], lhsT=ones_t[:], rhs=A[:], start=True, stop=True
            )
            e_sb = acc.tile([1, n_c], f32)
            nc.vector.tensor_copy(e_sb[:], e_ps[:])
            nc.sync.dma_start(out=energy[:], in_=e_sb[:])
    nc.finalize()
    return nc


def _softplus64(x: np.ndarray) -> np.ndarray:
    x = np.asarray(x, dtype=np.float64)
    return np.log1p(np.exp(-np.abs(x))) + np.maximum(x, 0.0)


def kernel(T, descriptor, query_feature, query_attention, irrep_weight_logit, ranges):
    descriptor = np.asarray(descriptor)
    query_feature = np.asarray(query_feature)
    a = np.maximum(np.asarray(query_attention, dtype=np.float64), 0.0)
    w_group = _softplus64(irrep_weight_logit) / (LN2 * G)
    w_feat = np.repeat(w_group, _GROUP_DIMS)
    c_qd = (BUMP * np.sqrt(2.0 * a[:, None] * w_feat[None, :])).astype(np.float32)

    # range mask: energy of out-of-range poses is the constant 1e5
    X = np.asarray(T, dtype=np.float32)[:, 4:7]
    rg = np.asarray(ranges, dtype=np.float32)
    in_range = np.all((rg[None, :, 1] >= X) & (X >= rg[None, :, 0]), axis=-1)
    idx = np.nonzero(in_range)[0]
    n = len(idx)

    n_c = max(1, -(-n // N_CORES))  # poses per core
    ppc = min(16, max(1, -(-n_c // 4)))  # poses per chunk
    nchunks = -(-n_c // ppc)
    n_c = ppc * nchunks
    n_pad = n_c * N_CORES

    # gather + quantize only the needed poses
    xs = np.zeros((n_pad, F_TOT), dtype=ml_dtypes.float8_e4m3)
    ys = np.zeros((n_pad, F_TOT), dtype=ml_dtypes.float8_e4m3)
    cf = c_qd.reshape(1, F_TOT)
    xs[:n] = np.clip(
        descriptor.reshape(NT, F_TOT)[idx] * cf, -240.0, 240.0
    ).astype(ml_dtypes.float8_e4m3)
    ys[:n] = np.clip(
        query_feature.reshape(NT, F_TOT)[idx] * cf, -240.0, 240.0
    ).astype(ml_dtypes.float8_e4m3)

    # remap: [n_pad, (s,p)] -> per core [p, chunk, (x|y), k, s]
    C = ppc * S_DIM
    xs = xs.reshape(N_CORES, nchunks, ppc, S_DIM, 128)
    ys = ys.reshape(N_CORES, nchunks, ppc, S_DIM, 128)
    z = np.stack([xs, ys], axis=2)  # [cores, chunks, 2, ppc, s, p]
    z = np.ascontiguousarray(np.moveaxis(z, 5, 2))  # [cores, chunks, p, 2, k, s]
    z = z.reshape(N_CORES, nchunks, 128, 2 * C)
    PEC = min(5120, (C // 1024) * 1024)
    z = np.concatenate(
        [z[..., PEC:C], z[..., C + PEC :], z[..., :PEC], z[..., C : C + PEC]],
        axis=-1,
    )
    z = np.ascontiguousarray(np.swapaxes(z, 1, 2)).reshape(
        N_CORES, 128, nchunks * 2 * C
    )

    smat = np.zeros((128, 2, 128), dtype=ml_dtypes.float8_e4m3)
    ii = np.arange(128)
    smat[ii, 0, ii] = 1.0
    smat[ii, 1, ii] = -1.0
    smat = smat.reshape(128, 256)
    onesv = np.full((128, 1), 1.0 / (BUMP * BUMP), dtype=np.float32)

    key = ("mask11", ppc, nchunks)
    nc = _cache.get(key)
    if nc is None:
        nc = _build(ppc, nchunks)
        _cache[key] = nc

    in_maps = [
        {"zin": z[i], "smat": smat, "onesv": onesv} for i in range(N_CORES)
    ]

    global _last_in_maps
    _last_in_maps = in_maps
    res = run_bass_kernel_spmd(nc, in_maps, core_ids=list(range(N_CORES)))
    e_sub = np.concatenate([r["energy"][0] for r in res.results])[:n]

    energy = np.full(NT, 100000.0, dtype=np.float32)
    energy[idx] = e_sub.astype(np.float32)
    return energy
